# revision 8
# baseline (speedup 1.0000x reference)
"""Trainium2 Bass kernel for chemprop-style BondMessagePassing (OMGNN_RNN).

Strategy (8 NeuronCores, SPMD):
- Nodes sharded 8 ways (6250/core). Each core carries edges incident to its
  nodes: "own" edges (dst local, grouped by 128-node window, dst-sorted) feed
  a one-hot-matmul segment_sum; "halo" edges (src local) make the reverse-edge
  lookup core-local. Per depth: node-level G = Magg @ Wh^T + b_h is computed
  per-shard and AllGathered (small); per-edge update H = relu(H0 + G[src] -
  H_prev[rev] @ Wh^T) uses two indirect-DMA row gathers; the edge-level matmul
  runs on PE via per-tile transposes. Activations stored bf16, accum fp32.
"""
import sys
sys.path.insert(0, "/opt/trn_rl_repo")
import numpy as np
import ml_dtypes

N_NODES, N_EDGES, NODE_DIM, BOND_DIM, HID, DEPTH = 50000, 500000, 160, 14, 160, 3

def _default_runner(nc, in_maps, core_ids, **kw):
    from concourse.bass_utils import run_bass_kernel_spmd as f
    return f(nc, in_maps, core_ids, **kw)

run_bass_kernel_spmd_ref = [_default_runner]
NC = 8
NPC = N_NODES // NC
WIN = 128
NWIN = (NPC + WIN - 1) // WIN          # 49
NPC_PAD = NWIN * WIN                   # 6272
BF = ml_dtypes.bfloat16


def _prep(x, edge_attr, edge_index, rev_edge_index):
    src = np.asarray(edge_index[0], np.int64)
    dst = np.asarray(edge_index[1], np.int64)
    rev = np.asarray(rev_edge_index, np.int64)
    owner_dst = dst // NPC
    owner_src = src // NPC
    per_core_win_edges, halo_lists = [], []
    for c in range(NC):
        own_mask = owner_dst == c
        halo_mask = (owner_src == c) & ~own_mask
        own_ids = np.nonzero(own_mask)[0]
        wloc = (dst[own_ids] - c * NPC) // WIN
        per_core_win_edges.append([own_ids[wloc == w] for w in range(NWIN)])
        halo_lists.append(np.nonzero(halo_mask)[0])
    TW = max(int(np.ceil(max(1, len(e)) / 128)) for wins in per_core_win_edges for e in wins)
    E_OWN = NWIN * TW * 128
    HMAX = max(len(h) for h in halo_lists)
    E_HALO = int(np.ceil(HMAX / 128)) * 128
    E_LOC = E_OWN + E_HALO
    meta = dict(TW=TW, E_OWN=E_OWN, E_HALO=E_HALO, E_LOC=E_LOC)
    cores = []
    for c in range(NC):
        gid = np.full(E_LOC, -1, np.int64)
        for w in range(NWIN):
            e = per_core_win_edges[c][w]
            gid[w * TW * 128: w * TW * 128 + len(e)] = e
        h = halo_lists[c]
        gid[E_OWN: E_OWN + len(h)] = h
        valid = gid >= 0
        l_of_g = np.full(N_EDGES, -1, np.int64)
        l_of_g[gid[valid]] = np.nonzero(valid)[0]
        revl = np.zeros(E_LOC, np.int64)
        rv = l_of_g[rev[gid[valid]]]
        assert (rv >= 0).all()
        revl[valid] = rv
        g = np.maximum(gid, 0)
        s = src[g]
        sidx = (s // NPC) * NPC_PAD + (s % NPC)
        sidx[~valid] = 0
        doff = np.full(E_LOC, 255, np.int64)
        ok = gid[:E_OWN] >= 0
        doff[:E_OWN][ok] = (dst[gid[:E_OWN][ok]] - c * NPC) % WIN
        xe = np.zeros((E_LOC, NODE_DIM + BOND_DIM + 1), np.float32)
        xe[:, :NODE_DIM] = x[s]
        xe[:, NODE_DIM:-1] = edge_attr[g]
        xe[:, -1] = 1.0
        xe[~valid] = 0.0
        xe[~valid, -1] = 1.0
        xo = np.zeros((NPC_PAD, NODE_DIM), np.float32)
        xo[:NPC] = x[c * NPC:(c + 1) * NPC]
        cores.append(dict(revl=revl, sidx=sidx, doff=doff, xe=xe, x_own=xo))
    return meta, cores


def kernel(x, edge_attr, edge_index, rev_edge_index, Wi_w, Wi_b, Wh_w, Wh_b, Wo_w, Wo_b):
    x = np.asarray(x, np.float32); edge_attr = np.asarray(edge_attr, np.float32)
    meta, cores = _prep(x, edge_attr, edge_index, rev_edge_index)
    TW, E_OWN, E_LOC = meta["TW"], meta["E_OWN"], meta["E_LOC"]
    T_ALL = E_LOC // 128
    T_OWN = E_OWN // 128

    from concourse import bass, bacc, mybir, tile
    from concourse.masks import make_identity
    run_bass_kernel_spmd = run_bass_kernel_spmd_ref[0]
    f32, bf16, i32 = mybir.dt.float32, mybir.dt.bfloat16, mybir.dt.int32

    nc = bacc.Bacc("TRN2", target_bir_lowering=False, debug=False, num_devices=NC)
    # ---- I/O ----
    xeT1 = nc.dram_tensor("xeT1", [128, E_LOC], bf16, kind="ExternalInput")
    xeT2 = nc.dram_tensor("xeT2", [47, E_LOC], bf16, kind="ExternalInput")
    srcidx_d = nc.dram_tensor("srcidx", [128, T_ALL], i32, kind="ExternalInput")
    revidx_d = nc.dram_tensor("revidx", [128, T_ALL], i32, kind="ExternalInput")
    doff_d = nc.dram_tensor("doff", [128, T_OWN], f32, kind="ExternalInput")
    WiT_d = nc.dram_tensor("WiT", [175, HID], bf16, kind="ExternalInput")
    WhT_d = nc.dram_tensor("WhT", [HID, HID], bf16, kind="ExternalInput")
    WoT_d = nc.dram_tensor("WoT", [321, HID], bf16, kind="ExternalInput")
    bh_d = nc.dram_tensor("bh", [128, HID], f32, kind="ExternalInput")
    xown_d = nc.dram_tensor("xown", [NPC_PAD, NODE_DIM], f32, kind="ExternalInput")
    y_d = nc.dram_tensor("y", [NC * NPC_PAD, HID], f32, kind="ExternalOutput")
    # ---- internals ----
    H0_d = nc.dram_tensor("H0", [E_LOC, HID], bf16)
    Hb_d = nc.dram_tensor("Hb", [E_LOC, HID], bf16)
    Gb_d = nc.dram_tensor("Gb", [NPC_PAD, HID], bf16)
    Gf_d = [nc.dram_tensor(f"Gf{k}", [NC * NPC_PAD, HID], bf16, addr_space="Shared") for k in range(2)]
    oo_d = nc.dram_tensor("oo", [NPC_PAD, HID], f32)
    of_d = nc.dram_tensor("of", [NC * NPC_PAD, HID], f32, addr_space="Shared")

    RG = [list(range(NC))]
    with tile.TileContext(nc) as tc:
        with tc.tile_pool(name="const", bufs=1) as cp, \
             tc.tile_pool(name="work", bufs=4) as wp, \
             tc.tile_pool(name="psum", bufs=2, space="PSUM") as pp, \
             tc.tile_pool(name="pwin", bufs=2, space="PSUM") as pwp:
            ident = cp.tile([128, 128], bf16)
            make_identity(nc, ident[:])
            iota = cp.tile([128, 128], f32)
            nc.gpsimd.iota(iota[:], pattern=[[1, 128]], channel_multiplier=0, allow_small_or_imprecise_dtypes=True)
            WiTa = cp.tile([128, HID], bf16); nc.sync.dma_start(out=WiTa[:], in_=WiT_d[0:128, :])
            WiTb = cp.tile([47, HID], bf16); nc.sync.dma_start(out=WiTb[:], in_=WiT_d[128:175, :])
            WhTa = cp.tile([128, HID], bf16); nc.sync.dma_start(out=WhTa[:], in_=WhT_d[0:128, :])
            WhTb = cp.tile([32, HID], bf16); nc.sync.dma_start(out=WhTb[:], in_=WhT_d[128:160, :])
            WoTc = []
            for ci, (a, b) in enumerate([(0, 128), (128, 256), (256, 321)]):
                w_ = cp.tile([b - a, HID], bf16, tag=f"wo{ci}")
                nc.sync.dma_start(out=w_[:], in_=WoT_d[a:b, :])
                WoTc.append(w_)
            bh = cp.tile([128, HID], f32); nc.sync.dma_start(out=bh[:], in_=bh_d[:])
            sidx = cp.tile([128, T_ALL], i32); nc.sync.dma_start(out=sidx[:], in_=srcidx_d[:])
            ridx = cp.tile([128, T_ALL], i32); nc.sync.dma_start(out=ridx[:], in_=revidx_d[:])
            doff = cp.tile([128, T_OWN], f32); nc.sync.dma_start(out=doff[:], in_=doff_d[:])

            def onehot(t):
                o = wp.tile([128, 128], bf16, tag="oh")
                nc.vector.tensor_scalar(out=o[:], in0=iota[:], scalar1=doff[:, t:t + 1],
                                        scalar2=None, op0=mybir.AluOpType.is_equal)
                return o

            def g_production(w, magg_psum, last_depth):
                """Window w done: PSUM [128,160] f32 -> G tile (or final out tile)."""
                magg = wp.tile([128, HID], f32, tag="magg")
                nc.vector.tensor_copy(out=magg[:], in_=magg_psum[:])
                if not last_depth:
                    mT1p = pp.tile([128, 128], bf16, tag="t1", space="PSUM")
                    mT2p = pp.tile([32, 128], bf16, tag="t2", space="PSUM")
                    mbf = wp.tile([128, HID], bf16, tag="mbf")
                    nc.vector.tensor_copy(out=mbf[:], in_=magg[:])
                    nc.tensor.transpose(out=mT1p[:], in_=mbf[:, 0:128], identity=ident[:])
                    nc.tensor.transpose(out=mT2p[:], in_=mbf[:, 128:160], identity=ident[:])
                    mT1 = wp.tile([128, 128], bf16, tag="mt1"); nc.vector.tensor_copy(out=mT1[:], in_=mT1p[:])
                    mT2 = wp.tile([32, 128], bf16, tag="mt2"); nc.vector.tensor_copy(out=mT2[:], in_=mT2p[:])
                    gp = pp.tile([128, HID], f32, tag="mm", space="PSUM")
                    nc.tensor.matmul(gp[:], lhsT=mT1[:], rhs=WhTa[:], start=True, stop=False)
                    nc.tensor.matmul(gp[:], lhsT=mT2[:], rhs=WhTb[:], start=False, stop=True)
                    gs = wp.tile([128, HID], bf16, tag="gs")
                    nc.vector.tensor_tensor(out=gs[:], in0=gp[:], in1=bh[:], op=mybir.AluOpType.add)
                    nc.sync.dma_start(out=Gb_d[w * 128:(w + 1) * 128, :], in_=gs[:])
                else:
                    rs = wp.tile([128, 1], f32, tag="rs")
                    nc.vector.tensor_reduce(out=rs[:], in_=magg[:], op=mybir.AluOpType.add,
                                            axis=mybir.AxisListType.X)
                    mask = wp.tile([128, 1], mybir.dt.uint8, tag="msk")
                    nc.vector.tensor_scalar(out=mask[:], in0=rs[:], scalar1=0.0, scalar2=None,
                                            op0=mybir.AluOpType.is_equal)
                    xo = wp.tile([128, NODE_DIM], f32, tag="xo")
                    nc.sync.dma_start(out=xo[:], in_=xown_d[w * 128:(w + 1) * 128, :])
                    m = wp.tile([128, HID], f32, tag="m")
                    nc.vector.select(out=m[:], mask=mask[:].to_broadcast([128, HID]),
                                     on_true=xo[:], on_false=magg[:])
                    xm = wp.tile([128, 321], bf16, tag="xm")
                    nc.vector.tensor_copy(out=xm[:, 0:NODE_DIM], in_=xo[:])
                    nc.vector.tensor_copy(out=xm[:, NODE_DIM:NODE_DIM + HID], in_=m[:])
                    nc.vector.memset(xm[:, 320:321], 1.0)
                    xT = []
                    for ci, (a, b) in enumerate([(0, 128), (128, 256), (256, 321)]):
                        tp = pp.tile([b - a if b - a <= 128 else 128, 128], bf16, tag="t1", space="PSUM")
                        nc.tensor.transpose(out=tp[:], in_=xm[:, a:b], identity=ident[:])
                        ts_ = wp.tile([b - a, 128], bf16, tag=f"xt{ci}")
                        nc.vector.tensor_copy(out=ts_[:], in_=tp[:])
                        xT.append((ts_, a, b))
                    op = pp.tile([128, HID], f32, tag="mm", space="PSUM")
                    for ci, (ts_, a, b) in enumerate(xT):
                        nc.tensor.matmul(op[:], lhsT=ts_[:], rhs=WoTc[ci][:],
                                         start=(ci == 0), stop=(ci == 2))
                    ot = wp.tile([128, HID], f32, tag="ot")
                    nc.scalar.activation(out=ot[:], in_=op[:], func=mybir.ActivationFunctionType.Relu)
                    nc.sync.dma_start(out=oo_d[w * 128:(w + 1) * 128, :], in_=ot[:])

            # ---------- Phase A: H0 + segsum0 + G0 ----------
            win_psum = None
            for t in range(T_ALL):
                xa = wp.tile([128, 128], bf16, tag="xa")
                nc.sync.dma_start(out=xa[:], in_=xeT1[:, t * 128:(t + 1) * 128])
                xb = wp.tile([47, 128], bf16, tag="xb")
                nc.sync.dma_start(out=xb[:], in_=xeT2[:, t * 128:(t + 1) * 128])
                hp = pp.tile([128, HID], f32, tag="mm", space="PSUM")
                nc.tensor.matmul(hp[:], lhsT=xa[:], rhs=WiTa[:], start=True, stop=False)
                nc.tensor.matmul(hp[:], lhsT=xb[:], rhs=WiTb[:], start=False, stop=True)
                h0 = wp.tile([128, HID], bf16, tag="h0")
                nc.scalar.activation(out=h0[:], in_=hp[:], func=mybir.ActivationFunctionType.Relu)
                nc.sync.dma_start(out=H0_d[t * 128:(t + 1) * 128, :], in_=h0[:])
                if t < T_OWN:
                    if t % TW == 0:
                        win_psum = pwp.tile([128, HID], f32, tag="win", space="PSUM")
                    o = onehot(t)
                    nc.tensor.matmul(win_psum[:], lhsT=o[:], rhs=h0[:],
                                     start=(t % TW == 0), stop=(t % TW == TW - 1))
                    if t % TW == TW - 1:
                        g_production(t // TW, win_psum, last_depth=False)
            nc.gpsimd.collective_compute("AllGather", mybir.AluOpType.bypass,
                                         replica_groups=RG, ins=[Gb_d[:]], outs=[Gf_d[0][:]])

            # ---------- Phases B/C: depth 1 and 2 ----------
            for k in (1, 2):
                last = (k == 2)
                Hsrc = H0_d if k == 1 else Hb_d
                Gsrc = Gf_d[k - 1]
                ntile = T_ALL if k == 1 else T_OWN
                for t in range(ntile):
                    gs = wp.tile([128, HID], bf16, tag="gG")
                    nc.gpsimd.indirect_dma_start(
                        out=gs[:], out_offset=None, in_=Gsrc[:, :],
                        in_offset=bass.IndirectOffsetOnAxis(ap=sidx[:, t:t + 1], axis=0))
                    hr = wp.tile([128, HID], bf16, tag="gH")
                    nc.gpsimd.indirect_dma_start(
                        out=hr[:], out_offset=None, in_=Hsrc[:, :],
                        in_offset=bass.IndirectOffsetOnAxis(ap=ridx[:, t:t + 1], axis=0))
                    t1p = pp.tile([128, 128], bf16, tag="t1", space="PSUM")
                    nc.tensor.transpose(out=t1p[:], in_=hr[:, 0:128], identity=ident[:])
                    t2p = pp.tile([32, 128], bf16, tag="t2", space="PSUM")
                    nc.tensor.transpose(out=t2p[:], in_=hr[:, 128:160], identity=ident[:])
                    t1 = wp.tile([128, 128], bf16, tag="t1s"); nc.vector.tensor_copy(out=t1[:], in_=t1p[:])
                    t2 = wp.tile([32, 128], bf16, tag="t2s"); nc.vector.tensor_copy(out=t2[:], in_=t2p[:])
                    qp = pp.tile([128, HID], f32, tag="mm", space="PSUM")
                    nc.tensor.matmul(qp[:], lhsT=t1[:], rhs=WhTa[:], start=True, stop=False)
                    nc.tensor.matmul(qp[:], lhsT=t2[:], rhs=WhTb[:], start=False, stop=True)
                    h0t = wp.tile([128, HID], bf16, tag="h0r")
                    nc.sync.dma_start(out=h0t[:], in_=H0_d[t * 128:(t + 1) * 128, :])
                    z = wp.tile([128, HID], f32, tag="z")
                    nc.vector.tensor_tensor(out=z[:], in0=gs[:], in1=qp[:], op=mybir.AluOpType.subtract)
                    z2 = wp.tile([128, HID], f32, tag="z2")
                    nc.vector.tensor_tensor(out=z2[:], in0=z[:], in1=h0t[:], op=mybir.AluOpType.add)
                    h = wp.tile([128, HID], bf16, tag="h")
                    nc.scalar.activation(out=h[:], in_=z2[:], func=mybir.ActivationFunctionType.Relu)
                    if not last:
                        nc.sync.dma_start(out=Hb_d[t * 128:(t + 1) * 128, :], in_=h[:])
                    if t < T_OWN:
                        if t % TW == 0:
                            win_psum = pwp.tile([128, HID], f32, tag="win", space="PSUM")
                        o = onehot(t)
                        nc.tensor.matmul(win_psum[:], lhsT=o[:], rhs=h[:],
                                         start=(t % TW == 0), stop=(t % TW == TW - 1))
                        if t % TW == TW - 1:
                            g_production(t // TW, win_psum, last_depth=last)
                if not last:
                    nc.gpsimd.collective_compute("AllGather", mybir.AluOpType.bypass,
                                                 replica_groups=RG, ins=[Gb_d[:]], outs=[Gf_d[1][:]])
            nc.gpsimd.collective_compute("AllGather", mybir.AluOpType.bypass,
                                         replica_groups=RG, ins=[oo_d[:]], outs=[of_d[:]])
            yt = wp.tile([128, HID], f32, tag="yt")
            for b in range(NC * NPC_PAD // 128):
                yt2 = wp.tile([128, HID], f32, tag="yt")
                nc.sync.dma_start(out=yt2[:], in_=of_d[b * 128:(b + 1) * 128, :])
                nc.sync.dma_start(out=y_d[b * 128:(b + 1) * 128, :], in_=yt2[:])
    nc.compile()

    Wi_aug = np.concatenate([np.asarray(Wi_w, np.float32).T, np.asarray(Wi_b, np.float32)[None, :]], 0)
    Wo_aug = np.concatenate([np.asarray(Wo_w, np.float32).T, np.asarray(Wo_b, np.float32)[None, :]], 0)
    WhT = np.asarray(Wh_w, np.float32).T
    bh_bc = np.tile(np.asarray(Wh_b, np.float32)[None, :], (128, 1))
    in_maps = []
    for c in range(NC):
        pc = cores[c]
        xeT = pc["xe"].T.astype(BF)                      # [175, E_LOC]
        si = pc["sidx"].reshape(T_ALL, 128).T.astype(np.int32)
        ri = pc["revl"].reshape(T_ALL, 128).T.astype(np.int32)
        do = pc["doff"][:E_OWN].reshape(T_OWN, 128).T.astype(np.float32)
        in_maps.append({
            "xeT1": np.ascontiguousarray(xeT[:128]), "xeT2": np.ascontiguousarray(xeT[128:175]),
            "srcidx": np.ascontiguousarray(si), "revidx": np.ascontiguousarray(ri),
            "doff": np.ascontiguousarray(do),
            "WiT": Wi_aug.astype(BF), "WhT": WhT.astype(BF), "WoT": Wo_aug.astype(BF),
            "bh": bh_bc, "xown": pc["x_own"],
        })
    res = run_bass_kernel_spmd(nc, in_maps, list(range(NC)))
    y = res.results[0]["y"].reshape(NC, NPC_PAD, HID)
    out = np.concatenate([y[c, :NPC] for c in range(NC)], 0)
    return out.astype(np.float32)


def _build_for_timing(x, edge_attr, edge_index, rev_edge_index, Wi_w, Wi_b, Wh_w, Wh_b, Wo_w, Wo_b):
    """Return (nc, in_maps) with the program compiled, for external timing."""
    import types
    holder = {}
    orig = run_bass_kernel_spmd_ref[0]
    def capture(nc, in_maps, core_ids, **kw):
        holder["nc"], holder["in_maps"] = nc, in_maps
        return orig(nc, in_maps, core_ids, **kw)
    run_bass_kernel_spmd_ref[0] = capture
    try:
        out = kernel(x, edge_attr, edge_index, rev_edge_index, Wi_w, Wi_b, Wh_w, Wh_b, Wo_w, Wo_b)
    finally:
        run_bass_kernel_spmd_ref[0] = orig
    return holder["nc"], holder["in_maps"], out


# revision 9
# speedup vs baseline: 1.7164x; 1.7164x over previous
"""Trainium2 Bass kernel for chemprop-style BondMessagePassing (OMGNN_RNN).

Strategy (8 NeuronCores, SPMD):
- Nodes sharded 8 ways (6250/core). Each core carries edges incident to its
  nodes: "own" edges (dst local, grouped by 128-node window, dst-sorted) feed
  a one-hot-matmul segment_sum; "halo" edges (src local) make the reverse-edge
  lookup core-local. Per depth: node-level G = Magg @ Wh^T + b_h is computed
  per-shard and AllGathered (small); per-edge update H = relu(H0 + G[src] -
  H_prev[rev] @ Wh^T) uses two indirect-DMA row gathers; the edge-level matmul
  runs on PE via per-tile transposes. Activations stored bf16, accum fp32.
"""
import sys
sys.path.insert(0, "/opt/trn_rl_repo")
import numpy as np
import ml_dtypes

N_NODES, N_EDGES, NODE_DIM, BOND_DIM, HID, DEPTH = 50000, 500000, 160, 14, 160, 3

def _default_runner(nc, in_maps, core_ids, **kw):
    from concourse.bass_utils import run_bass_kernel_spmd as f
    return f(nc, in_maps, core_ids, **kw)

run_bass_kernel_spmd_ref = [_default_runner]
NC = 8
NPC = N_NODES // NC
WIN = 128
NWIN = (NPC + WIN - 1) // WIN          # 49
NPC_PAD = NWIN * WIN                   # 6272
BF = ml_dtypes.bfloat16


def _prep(x, edge_attr, edge_index, rev_edge_index):
    src = np.asarray(edge_index[0], np.int64)
    dst = np.asarray(edge_index[1], np.int64)
    rev = np.asarray(rev_edge_index, np.int64)
    owner_dst = dst // NPC
    owner_src = src // NPC
    per_core_win_edges, halo_lists = [], []
    for c in range(NC):
        own_mask = owner_dst == c
        halo_mask = (owner_src == c) & ~own_mask
        own_ids = np.nonzero(own_mask)[0]
        wloc = (dst[own_ids] - c * NPC) // WIN
        per_core_win_edges.append([own_ids[wloc == w] for w in range(NWIN)])
        halo_lists.append(np.nonzero(halo_mask)[0])
    TW = max(int(np.ceil(max(1, len(e)) / 128)) for wins in per_core_win_edges for e in wins)
    E_OWN = NWIN * TW * 128
    HMAX = max(len(h) for h in halo_lists)
    E_HALO = int(np.ceil(HMAX / 128)) * 128
    E_LOC = E_OWN + E_HALO
    meta = dict(TW=TW, E_OWN=E_OWN, E_HALO=E_HALO, E_LOC=E_LOC)
    cores = []
    for c in range(NC):
        gid = np.full(E_LOC, -1, np.int64)
        for w in range(NWIN):
            e = per_core_win_edges[c][w]
            gid[w * TW * 128: w * TW * 128 + len(e)] = e
        h = halo_lists[c]
        gid[E_OWN: E_OWN + len(h)] = h
        valid = gid >= 0
        l_of_g = np.full(N_EDGES, -1, np.int64)
        l_of_g[gid[valid]] = np.nonzero(valid)[0]
        revl = np.zeros(E_LOC, np.int64)
        rv = l_of_g[rev[gid[valid]]]
        assert (rv >= 0).all()
        revl[valid] = rv
        g = np.maximum(gid, 0)
        s = src[g]
        sidx = (s // NPC) * NPC_PAD + (s % NPC)
        sidx[~valid] = 0
        doff = np.full(E_LOC, 255, np.int64)
        ok = gid[:E_OWN] >= 0
        doff[:E_OWN][ok] = (dst[gid[:E_OWN][ok]] - c * NPC) % WIN
        xe = np.zeros((E_LOC, NODE_DIM + BOND_DIM + 1), np.float32)
        xe[:, :NODE_DIM] = x[s]
        xe[:, NODE_DIM:-1] = edge_attr[g]
        xe[:, -1] = 1.0
        xe[~valid] = 0.0
        xe[~valid, -1] = 1.0
        xo = np.zeros((NPC_PAD, NODE_DIM), np.float32)
        xo[:NPC] = x[c * NPC:(c + 1) * NPC]
        cores.append(dict(revl=revl, sidx=sidx, doff=doff, xe=xe, x_own=xo))
    return meta, cores


def kernel(x, edge_attr, edge_index, rev_edge_index, Wi_w, Wi_b, Wh_w, Wh_b, Wo_w, Wo_b):
    x = np.asarray(x, np.float32); edge_attr = np.asarray(edge_attr, np.float32)
    meta, cores = _prep(x, edge_attr, edge_index, rev_edge_index)
    TW, E_OWN, E_LOC = meta["TW"], meta["E_OWN"], meta["E_LOC"]
    T_ALL = E_LOC // 128
    T_OWN = E_OWN // 128

    from concourse import bass, bacc, mybir, tile
    from concourse.masks import make_identity
    run_bass_kernel_spmd = run_bass_kernel_spmd_ref[0]
    f32, bf16, i32 = mybir.dt.float32, mybir.dt.bfloat16, mybir.dt.int32

    nc = bacc.Bacc("TRN2", target_bir_lowering=False, debug=False, num_devices=NC)
    # ---- I/O ----
    xeT1 = nc.dram_tensor("xeT1", [128, E_LOC], bf16, kind="ExternalInput")
    xeT2 = nc.dram_tensor("xeT2", [47, E_LOC], bf16, kind="ExternalInput")
    srcidx_d = nc.dram_tensor("srcidx", [128, T_ALL], i32, kind="ExternalInput")
    revidx_d = nc.dram_tensor("revidx", [128, T_ALL], i32, kind="ExternalInput")
    doff_d = nc.dram_tensor("doff", [128, T_OWN], f32, kind="ExternalInput")
    WiT_d = nc.dram_tensor("WiT", [175, HID], bf16, kind="ExternalInput")
    WhT_d = nc.dram_tensor("WhT", [HID, HID], bf16, kind="ExternalInput")
    WoT_d = nc.dram_tensor("WoT", [321, HID], bf16, kind="ExternalInput")
    bh_d = nc.dram_tensor("bh", [128, HID], f32, kind="ExternalInput")
    xown_d = nc.dram_tensor("xown", [NPC_PAD, NODE_DIM], f32, kind="ExternalInput")
    y_d = nc.dram_tensor("y", [NC * NPC_PAD, HID], f32, kind="ExternalOutput")
    # ---- internals ----
    H0_d = nc.dram_tensor("H0", [E_LOC, HID], bf16)
    Hb_d = nc.dram_tensor("Hb", [E_LOC, HID], bf16)
    Gb_d = nc.dram_tensor("Gb", [NPC_PAD, HID], bf16)
    Gf_d = [nc.dram_tensor(f"Gf{k}", [NC * NPC_PAD, HID], bf16, addr_space="Shared") for k in range(2)]
    oo_d = nc.dram_tensor("oo", [NPC_PAD, HID], f32)
    of_d = nc.dram_tensor("of", [NC * NPC_PAD, HID], f32, addr_space="Shared")

    RG = [list(range(NC))]
    with tile.TileContext(nc) as tc:
        with tc.tile_pool(name="const", bufs=1) as cp, \
             tc.tile_pool(name="work", bufs=4) as wp, \
             tc.tile_pool(name="gath", bufs=10) as gp_pool, \
             tc.tile_pool(name="psum", bufs=2, space="PSUM") as pp, \
             tc.tile_pool(name="pwin", bufs=2, space="PSUM") as pwp:
            ident = cp.tile([128, 128], bf16)
            make_identity(nc, ident[:])
            iota = cp.tile([128, 128], f32)
            nc.gpsimd.iota(iota[:], pattern=[[1, 128]], channel_multiplier=0, allow_small_or_imprecise_dtypes=True)
            WiTa = cp.tile([128, HID], bf16); nc.sync.dma_start(out=WiTa[:], in_=WiT_d[0:128, :])
            WiTb = cp.tile([47, HID], bf16); nc.sync.dma_start(out=WiTb[:], in_=WiT_d[128:175, :])
            WhTa = cp.tile([128, HID], bf16); nc.sync.dma_start(out=WhTa[:], in_=WhT_d[0:128, :])
            WhTb = cp.tile([32, HID], bf16); nc.sync.dma_start(out=WhTb[:], in_=WhT_d[128:160, :])
            WoTc = []
            for ci, (a, b) in enumerate([(0, 128), (128, 256), (256, 321)]):
                w_ = cp.tile([b - a, HID], bf16, tag=f"wo{ci}")
                nc.sync.dma_start(out=w_[:], in_=WoT_d[a:b, :])
                WoTc.append(w_)
            bh = cp.tile([128, HID], f32); nc.sync.dma_start(out=bh[:], in_=bh_d[:])
            sidx = cp.tile([128, T_ALL], i32); nc.sync.dma_start(out=sidx[:], in_=srcidx_d[:])
            ridx = cp.tile([128, T_ALL], i32); nc.sync.dma_start(out=ridx[:], in_=revidx_d[:])
            doff = cp.tile([128, T_OWN], f32); nc.sync.dma_start(out=doff[:], in_=doff_d[:])

            def onehot(t):
                o = wp.tile([128, 128], bf16, tag="oh")
                nc.vector.tensor_scalar(out=o[:], in0=iota[:], scalar1=doff[:, t:t + 1],
                                        scalar2=None, op0=mybir.AluOpType.is_equal)
                return o

            def g_production(w, magg_psum, last_depth):
                """Window w done: PSUM [128,160] f32 -> G tile (or final out tile)."""
                magg = wp.tile([128, HID], f32, tag="magg")
                nc.vector.tensor_copy(out=magg[:], in_=magg_psum[:])
                if not last_depth:
                    mT1p = pp.tile([128, 128], bf16, tag="t1", space="PSUM")
                    mT2p = pp.tile([32, 128], bf16, tag="t2", space="PSUM")
                    mbf = wp.tile([128, HID], bf16, tag="mbf")
                    nc.vector.tensor_copy(out=mbf[:], in_=magg[:])
                    nc.tensor.transpose(out=mT1p[:], in_=mbf[:, 0:128], identity=ident[:])
                    nc.tensor.transpose(out=mT2p[:], in_=mbf[:, 128:160], identity=ident[:])
                    mT1 = wp.tile([128, 128], bf16, tag="mt1"); nc.vector.tensor_copy(out=mT1[:], in_=mT1p[:])
                    mT2 = wp.tile([32, 128], bf16, tag="mt2"); nc.vector.tensor_copy(out=mT2[:], in_=mT2p[:])
                    gp = pp.tile([128, HID], f32, tag="mm", space="PSUM")
                    nc.tensor.matmul(gp[:], lhsT=mT1[:], rhs=WhTa[:], start=True, stop=False)
                    nc.tensor.matmul(gp[:], lhsT=mT2[:], rhs=WhTb[:], start=False, stop=True)
                    gs = wp.tile([128, HID], bf16, tag="gs")
                    nc.vector.tensor_tensor(out=gs[:], in0=gp[:], in1=bh[:], op=mybir.AluOpType.add)
                    nc.sync.dma_start(out=Gb_d[w * 128:(w + 1) * 128, :], in_=gs[:])
                else:
                    rs = wp.tile([128, 1], f32, tag="rs")
                    nc.vector.tensor_reduce(out=rs[:], in_=magg[:], op=mybir.AluOpType.add,
                                            axis=mybir.AxisListType.X)
                    mask = wp.tile([128, 1], mybir.dt.uint8, tag="msk")
                    nc.vector.tensor_scalar(out=mask[:], in0=rs[:], scalar1=0.0, scalar2=None,
                                            op0=mybir.AluOpType.is_equal)
                    xo = wp.tile([128, NODE_DIM], f32, tag="xo")
                    nc.sync.dma_start(out=xo[:], in_=xown_d[w * 128:(w + 1) * 128, :])
                    m = wp.tile([128, HID], f32, tag="m")
                    nc.vector.select(out=m[:], mask=mask[:].to_broadcast([128, HID]),
                                     on_true=xo[:], on_false=magg[:])
                    xm = wp.tile([128, 321], bf16, tag="xm")
                    nc.vector.tensor_copy(out=xm[:, 0:NODE_DIM], in_=xo[:])
                    nc.vector.tensor_copy(out=xm[:, NODE_DIM:NODE_DIM + HID], in_=m[:])
                    nc.vector.memset(xm[:, 320:321], 1.0)
                    xT = []
                    for ci, (a, b) in enumerate([(0, 128), (128, 256), (256, 321)]):
                        tp = pp.tile([b - a if b - a <= 128 else 128, 128], bf16, tag="t1", space="PSUM")
                        nc.tensor.transpose(out=tp[:], in_=xm[:, a:b], identity=ident[:])
                        ts_ = wp.tile([b - a, 128], bf16, tag=f"xt{ci}")
                        nc.vector.tensor_copy(out=ts_[:], in_=tp[:])
                        xT.append((ts_, a, b))
                    op = pp.tile([128, HID], f32, tag="mm", space="PSUM")
                    for ci, (ts_, a, b) in enumerate(xT):
                        nc.tensor.matmul(op[:], lhsT=ts_[:], rhs=WoTc[ci][:],
                                         start=(ci == 0), stop=(ci == 2))
                    ot = wp.tile([128, HID], f32, tag="ot")
                    nc.scalar.activation(out=ot[:], in_=op[:], func=mybir.ActivationFunctionType.Relu)
                    nc.sync.dma_start(out=oo_d[w * 128:(w + 1) * 128, :], in_=ot[:])

            # ---------- Phase A: H0 + segsum0 + G0 ----------
            win_psum = None
            for t in range(T_ALL):
                xa = gp_pool.tile([128, 128], bf16, tag="xa")
                nc.sync.dma_start(out=xa[:], in_=xeT1[:, t * 128:(t + 1) * 128])
                xb = gp_pool.tile([47, 128], bf16, tag="xb")
                nc.sync.dma_start(out=xb[:], in_=xeT2[:, t * 128:(t + 1) * 128])
                hp = pp.tile([128, HID], f32, tag="mm", space="PSUM")
                nc.tensor.matmul(hp[:], lhsT=xa[:], rhs=WiTa[:], start=True, stop=False)
                nc.tensor.matmul(hp[:], lhsT=xb[:], rhs=WiTb[:], start=False, stop=True)
                h0 = wp.tile([128, HID], bf16, tag="h0")
                nc.scalar.activation(out=h0[:], in_=hp[:], func=mybir.ActivationFunctionType.Relu)
                nc.sync.dma_start(out=H0_d[t * 128:(t + 1) * 128, :], in_=h0[:])
                if t < T_OWN:
                    if t % TW == 0:
                        win_psum = pwp.tile([128, HID], f32, tag="win", space="PSUM")
                    o = onehot(t)
                    nc.tensor.matmul(win_psum[:], lhsT=o[:], rhs=h0[:],
                                     start=(t % TW == 0), stop=(t % TW == TW - 1))
                    if t % TW == TW - 1:
                        g_production(t // TW, win_psum, last_depth=False)
            nc.gpsimd.collective_compute("AllGather", mybir.AluOpType.bypass,
                                         replica_groups=RG, ins=[Gb_d[:]], outs=[Gf_d[0][:]])

            # ---------- Phases B/C: depth 1 and 2 ----------
            for k in (1, 2):
                last = (k == 2)
                Hsrc = H0_d if k == 1 else Hb_d
                Gsrc = Gf_d[k - 1]
                ntile = T_ALL if k == 1 else T_OWN
                for t in range(ntile):
                    gs = gp_pool.tile([128, HID], bf16, tag="gG")
                    nc.gpsimd.indirect_dma_start(
                        out=gs[:], out_offset=None, in_=Gsrc[:, :],
                        in_offset=bass.IndirectOffsetOnAxis(ap=sidx[:, t:t + 1], axis=0))
                    hr = gp_pool.tile([128, HID], bf16, tag="gH")
                    nc.gpsimd.indirect_dma_start(
                        out=hr[:], out_offset=None, in_=Hsrc[:, :],
                        in_offset=bass.IndirectOffsetOnAxis(ap=ridx[:, t:t + 1], axis=0))
                    t1p = pp.tile([128, 128], bf16, tag="t1", space="PSUM")
                    nc.tensor.transpose(out=t1p[:], in_=hr[:, 0:128], identity=ident[:])
                    t2p = pp.tile([32, 128], bf16, tag="t2", space="PSUM")
                    nc.tensor.transpose(out=t2p[:], in_=hr[:, 128:160], identity=ident[:])
                    t1 = wp.tile([128, 128], bf16, tag="t1s"); nc.vector.tensor_copy(out=t1[:], in_=t1p[:])
                    t2 = wp.tile([32, 128], bf16, tag="t2s"); nc.vector.tensor_copy(out=t2[:], in_=t2p[:])
                    qp = pp.tile([128, HID], f32, tag="mm", space="PSUM")
                    nc.tensor.matmul(qp[:], lhsT=t1[:], rhs=WhTa[:], start=True, stop=False)
                    nc.tensor.matmul(qp[:], lhsT=t2[:], rhs=WhTb[:], start=False, stop=True)
                    h0t = gp_pool.tile([128, HID], bf16, tag="h0r")
                    nc.sync.dma_start(out=h0t[:], in_=H0_d[t * 128:(t + 1) * 128, :])
                    z = wp.tile([128, HID], f32, tag="z")
                    nc.vector.tensor_tensor(out=z[:], in0=gs[:], in1=qp[:], op=mybir.AluOpType.subtract)
                    z2 = wp.tile([128, HID], f32, tag="z2")
                    nc.vector.tensor_tensor(out=z2[:], in0=z[:], in1=h0t[:], op=mybir.AluOpType.add)
                    h = wp.tile([128, HID], bf16, tag="h")
                    nc.scalar.activation(out=h[:], in_=z2[:], func=mybir.ActivationFunctionType.Relu)
                    if not last:
                        nc.sync.dma_start(out=Hb_d[t * 128:(t + 1) * 128, :], in_=h[:])
                    if t < T_OWN:
                        if t % TW == 0:
                            win_psum = pwp.tile([128, HID], f32, tag="win", space="PSUM")
                        o = onehot(t)
                        nc.tensor.matmul(win_psum[:], lhsT=o[:], rhs=h[:],
                                         start=(t % TW == 0), stop=(t % TW == TW - 1))
                        if t % TW == TW - 1:
                            g_production(t // TW, win_psum, last_depth=last)
                if not last:
                    nc.gpsimd.collective_compute("AllGather", mybir.AluOpType.bypass,
                                                 replica_groups=RG, ins=[Gb_d[:]], outs=[Gf_d[1][:]])
            nc.gpsimd.collective_compute("AllGather", mybir.AluOpType.bypass,
                                         replica_groups=RG, ins=[oo_d[:]], outs=[of_d[:]])
            nc.sync.dma_start(out=y_d[:], in_=of_d[:])
    nc.compile()

    Wi_aug = np.concatenate([np.asarray(Wi_w, np.float32).T, np.asarray(Wi_b, np.float32)[None, :]], 0)
    Wo_aug = np.concatenate([np.asarray(Wo_w, np.float32).T, np.asarray(Wo_b, np.float32)[None, :]], 0)
    WhT = np.asarray(Wh_w, np.float32).T
    bh_bc = np.tile(np.asarray(Wh_b, np.float32)[None, :], (128, 1))
    in_maps = []
    for c in range(NC):
        pc = cores[c]
        xeT = pc["xe"].T.astype(BF)                      # [175, E_LOC]
        si = pc["sidx"].reshape(T_ALL, 128).T.astype(np.int32)
        ri = pc["revl"].reshape(T_ALL, 128).T.astype(np.int32)
        do = pc["doff"][:E_OWN].reshape(T_OWN, 128).T.astype(np.float32)
        in_maps.append({
            "xeT1": np.ascontiguousarray(xeT[:128]), "xeT2": np.ascontiguousarray(xeT[128:175]),
            "srcidx": np.ascontiguousarray(si), "revidx": np.ascontiguousarray(ri),
            "doff": np.ascontiguousarray(do),
            "WiT": Wi_aug.astype(BF), "WhT": WhT.astype(BF), "WoT": Wo_aug.astype(BF),
            "bh": bh_bc, "xown": pc["x_own"],
        })
    res = run_bass_kernel_spmd(nc, in_maps, list(range(NC)))
    y = res.results[0]["y"].reshape(NC, NPC_PAD, HID)
    out = np.concatenate([y[c, :NPC] for c in range(NC)], 0)
    return out.astype(np.float32)


def _build_for_timing(x, edge_attr, edge_index, rev_edge_index, Wi_w, Wi_b, Wh_w, Wh_b, Wo_w, Wo_b):
    """Return (nc, in_maps) with the program compiled, for external timing."""
    import types
    holder = {}
    orig = run_bass_kernel_spmd_ref[0]
    def capture(nc, in_maps, core_ids, **kw):
        holder["nc"], holder["in_maps"] = nc, in_maps
        return orig(nc, in_maps, core_ids, **kw)
    run_bass_kernel_spmd_ref[0] = capture
    try:
        out = kernel(x, edge_attr, edge_index, rev_edge_index, Wi_w, Wi_b, Wh_w, Wh_b, Wo_w, Wo_b)
    finally:
        run_bass_kernel_spmd_ref[0] = orig
    return holder["nc"], holder["in_maps"], out
